# revision 13
# baseline (speedup 1.0000x reference)
"""Trainium2 Bass kernel for nn_AttentionLayer (B=2, L=S=2048, D=1024, H=16).

Sharding: batch x head-group. Core c handles batch b=c//4 and heads
[4*(c%4), 4*(c%4)+4). Column-parallel Wq/Wk/Wv, row-parallel We; the
per-core partial outputs are summed on the host (4 partials per batch).

v2 design (vs baseline): all on-chip data bf16 (half DMA + FWL weight
loads), score matmuls row-tiled two heads at a time (K=64 each on row
groups 0/64 -> concurrent), output projection contracts K=128 with two
heads stacked on the partition dim, kT/v held as per-chunk tiles so
attention starts before projections finish, and softmax normalization
uses the DRAM-bounce partition broadcast (no PE/PSUM involvement).
PSUM: sc 2x2 banks + av 2 + proj/fin 2 = 8.

Host folds the zero-cost pieces: bv and be shift every output row by
(bv @ We + be) because softmax rows sum to 1; bq/bk applied on-device.
"""
import sys

for _p in ("/opt/trn_rl_repo", "/root/.axon_site/_ro/trn_rl_repo"):
    if _p not in sys.path:
        sys.path.insert(0, _p)

import ml_dtypes
import numpy as np

import concourse.bass as bass
import concourse.mybir as mybir
from concourse import bacc
from concourse.bass import AP
from concourse.tile import TileContext

F32 = mybir.dt.float32
BF16 = mybir.dt.bfloat16
BF16NP = ml_dtypes.bfloat16

D = 1024          # model dim
H_TOTAL = 16
HG = 4            # heads per core
E = 64            # head dim
M = HG * E        # 256 projected cols per core
DT = D // 128     # 8 d-tiles
LCH = 512         # l-chunk
B = 2
N_CORES = 8


def build_program(L=2048, S=2048, sg=2, sc_bufs=1, ex_bufs=3, stream_bufs=3,
                  ob_bufs=2, o_bufs=2, n_bufs=2, av_bufs=3, proj_bufs=1):
    nc = bacc.Bacc("TRN2")
    QT = nc.dram_tensor("QT", [D, L], BF16, kind="ExternalInput")
    KT = nc.dram_tensor("KT", [D, S], BF16, kind="ExternalInput")
    VT = nc.dram_tensor("VT", [D, S], BF16, kind="ExternalInput")
    WQ = nc.dram_tensor("WQ", [D, M], BF16, kind="ExternalInput")
    WK = nc.dram_tensor("WK", [D, M], BF16, kind="ExternalInput")
    WV = nc.dram_tensor("WV", [D, M], BF16, kind="ExternalInput")
    WE = nc.dram_tensor("WE", [128, 2, D], BF16, kind="ExternalInput")
    BQ = nc.dram_tensor("BQ", [2, 128], F32, kind="ExternalInput")
    BK = nc.dram_tensor("BK", [2, 128], F32, kind="ExternalInput")
    OUT = nc.dram_tensor("OUT", [L, D], F32, kind="ExternalOutput")
    RB = nc.dram_tensor("RB", [(L // LCH) * HG, LCH], F32, kind="Internal")

    n_lch = L // LCH
    n_st = S // 128
    n_ch = S // LCH          # kT/v chunks
    st_per_ch = LCH // 128   # 4 s-tiles per chunk
    n_g = n_st // sg
    EXP = mybir.ActivationFunctionType.Exp

    with TileContext(nc) as tc:
        with tc.tile_pool(name="const", bufs=1) as cpool, \
             tc.tile_pool(name="stream", bufs=stream_bufs) as spool, \
             tc.tile_pool(name="ex", bufs=ex_bufs) as expool, \
             tc.tile_pool(name="norm", bufs=n_bufs) as npool, \
             tc.tile_pool(name="outw", bufs=o_bufs) as opool, \
             tc.tile_pool(name="ob", bufs=ob_bufs) as obpool, \
             tc.tile_pool(name="psc", bufs=sc_bufs, space="PSUM") as scpool, \
             tc.tile_pool(name="pav", bufs=av_bufs, space="PSUM") as avpool, \
             tc.tile_pool(name="pproj", bufs=proj_bufs, space="PSUM") as ppool:

            wq_sb = cpool.tile([128, DT, M], BF16, tag="wq")
            wk_sb = cpool.tile([128, DT, M], BF16, tag="wk")
            wv_sb = cpool.tile([128, DT, M], BF16, tag="wv")
            we_sb = cpool.tile([128, 2, D], BF16, tag="we")
            bq_sb = cpool.tile([128, 2], F32, tag="bq")
            bk_sb = cpool.tile([128, 2], F32, tag="bk")
            nc.sync.dma_start(wq_sb[:, :, :], WQ.rearrange("(t p) m -> p t m", p=128))
            nc.sync.dma_start(wk_sb[:, :, :], WK.rearrange("(t p) m -> p t m", p=128))
            nc.sync.dma_start(wv_sb[:, :, :], WV.rearrange("(t p) m -> p t m", p=128))
            nc.sync.dma_start(we_sb[:, :, :], WE[:, :, :])
            nc.sync.dma_start(bq_sb[:, :], BQ.rearrange("t p -> p t"))
            nc.sync.dma_start(bk_sb[:, :], BK.rearrange("t p -> p t"))

            # per-l-chunk qT tiles; per-s-chunk kT and v tiles (chunk
            # granularity lets attention start as soon as the first
            # chunks are projected).
            qT_t = [cpool.tile([128, 2, LCH], BF16, tag=f"qT{i}",
                               name=f"qT{i}") for i in range(n_lch)]
            kT_t = [cpool.tile([128, 2, LCH], BF16, tag=f"kT{i}",
                               name=f"kT{i}") for i in range(n_ch)]
            # 96 cols per head: 64 value cols + 32 ones-columns, so the
            # AV matmul lands the softmax denominator replicated on PSUM
            # rows 64:96 (feeds the 32-partition transposed reciprocal).
            v_t = [cpool.tile([128, st_per_ch, HG * 96], BF16, tag=f"v{i}",
                              name=f"v{i}") for i in range(n_ch)]
            for vt in v_t:
                nc.vector.memset(vt[:, :, :], 1.0)

            # ---- projections ----
            def emit_qk_proj_chunk(XT, w_sb, b_sb, dst, c):
                xtr = XT.rearrange("(t p) l -> p t l", p=128)
                ch = spool.tile([128, DT, LCH], BF16, tag="ch")
                nc.sync.dma_start(ch[:, :, :],
                                  xtr[:, :, c * LCH:(c + 1) * LCH])
                for mt in range(2):
                    ps = ppool.tile([128, LCH], F32, tag="proj")
                    for dt in range(DT):
                        nc.tensor.matmul(
                            ps[:, :],
                            w_sb[:, dt, mt * 128:(mt + 1) * 128],
                            ch[:, dt, :],
                            start=(dt == 0), stop=(dt == DT - 1),
                        )
                    nc.vector.tensor_scalar_add(
                        dst[:, mt, :], ps[:, :], b_sb[:, mt:mt + 1],
                    )

            def emit_v_proj_chunk(c):
                vtr = VT.rearrange("(t p) s -> p t s", p=128)
                ch = spool.tile([128, DT, LCH], BF16, tag="ch")
                nc.sync.dma_start(ch[:, :, :],
                                  vtr[:, :, c * LCH:(c + 1) * LCH])
                for st4 in range(st_per_ch):
                    ps = ppool.tile([128, M], F32, tag="proj")
                    for dt in range(DT):
                        nc.tensor.matmul(
                            ps[:, :],
                            ch[:, dt, st4 * 128:(st4 + 1) * 128],
                            wv_sb[:, dt, :],
                            start=(dt == 0), stop=(dt == DT - 1),
                        )
                    dstv = v_t[c][:, st4, :].rearrange(
                        "p (h c) -> p h c", c=96)[:, :, 0:64]
                    srcv = ps.rearrange("p (h c) -> p h c", c=64)
                    nc.vector.tensor_copy(dstv, srcv)

            # Q chunk 0 first (PE warm-up while K/V DMA streams), then
            # K and V interleaved by chunk so attention's s-tile 0..3
            # deps clear early.
            emit_qk_proj_chunk(QT, wq_sb, bq_sb, qT_t[0], 0)
            for c in range(n_ch):
                emit_qk_proj_chunk(KT, wk_sb, bk_sb, kT_t[c], c)
                emit_v_proj_chunk(c)

            # ---- attention + output projection ----
            for lc in range(n_lch):
                # project the NEXT l-chunk's queries here: the scheduler
                # uses these matmuls as PE filler while exp runs
                if lc + 1 < n_lch:
                    emit_qk_proj_chunk(QT, wq_sb, bq_sb, qT_t[lc + 1], lc + 1)
                outT = opool.tile([128, 2, LCH], BF16, tag="outT")
                for hp in range(HG // 2):
                    h0, h1 = 2 * hp, 2 * hp + 1
                    avs = {}
                    for h in (h0, h1):
                        avs[h] = avpool.tile([128, LCH], F32, tag="av",
                                             name=f"av_{lc}_{h}")
                    for g in range(n_g):
                        # one 4-bank unit: sg s-tiles x 2 heads; a single
                        # 2048-elem exp call covers the whole unit
                        scu = scpool.tile([128, sg, 2, LCH], F32, tag="sc",
                                          name=f"sc_{lc}_{hp}_{g}")
                        for j in range(sg):
                            st = g * sg + j
                            c, s4 = st // st_per_ch, st % st_per_ch
                            sl = slice(s4 * 128, (s4 + 1) * 128)
                            # two heads row-tiled: K=64 each on row
                            # groups 0 / 64, concurrent in the array
                            nc.tensor.matmul(
                                scu[:, j, 0, :],
                                kT_t[c][0:64, hp, sl],
                                qT_t[lc][0:64, hp, :],
                                start=True, stop=True,
                                tile_position=(0, 0),
                            )
                            nc.tensor.matmul(
                                scu[:, j, 1, :],
                                kT_t[c][64:128, hp, sl],
                                qT_t[lc][64:128, hp, :],
                                start=True, stop=True,
                                tile_position=(64, 0),
                            )
                        ex = expool.tile([128, sg, 2, LCH], BF16, tag="ex",
                                         name=f"ex_{lc}_{hp}_{g}")
                        nc.scalar.activation(
                            ex[:, :, :, :], scu[:, :, :, :], EXP,
                            bias=0.0, scale=0.125,
                        )
                        for h in (h0, h1):
                            for j in range(sg):
                                st = g * sg + j
                                c, s4 = st // st_per_ch, st % st_per_ch
                                nc.tensor.matmul(
                                    avs[h][0:96, :],
                                    v_t[c][:, s4, h * 96:(h + 1) * 96],
                                    ex[:, j, h - h0, :],
                                    start=(st == 0),
                                    stop=(st == n_st - 1),
                                )
                    for h in (h0, h1):
                        av = avs[h]
                        # denominator sits replicated on av rows 64:96.
                        # 32x32 block-transpose spreads l across 32
                        # partitions: t1[p, b, f] = denom[32b + p], so
                        # reciprocal runs on 16 elems/lane instead of 512.
                        t1 = npool.tile([32, n_st, 32], F32, tag="t1",
                                        name=f"t1_{lc}_{h}")
                        nc.vector.transpose(t1[:, :, :], av[64:96, :])
                        rc = npool.tile([32, n_st], F32, tag="rc",
                                        name=f"rc_{lc}_{h}")
                        nc.vector.reciprocal(rc[:, :], t1[:, :, 0])
                        # bounce through DRAM: write order (p, b) =
                        # RB[16p + b] = 1/denom[32b + p]; broadcast read
                        # back with the matching affine pattern.
                        idx = lc * HG + h
                        rb = RB[idx, :]
                        # scatter-write so DRAM holds recip in l-order:
                        # RB[32b + p] = rc[p, b]
                        nc.sync.dma_start(
                            AP(rb.tensor, rb.offset, [[1, 32], [32, n_st]]),
                            rc[:, :])
                        rep = npool.tile([64, LCH], F32, tag="rep",
                                         name=f"rep_{lc}_{h}")
                        nc.sync.dma_start(
                            rep[:, :],
                            AP(rb.tensor, rb.offset, [[0, 64], [1, LCH]]))
                        p0 = 64 * (h % 2)
                        nc.vector.tensor_mul(
                            outT[p0:p0 + 64, h // 2, :], av[0:64, :],
                            rep[:, :],
                        )
                for ls in range(LCH // 128):
                    ob = obpool.tile([128, D], F32, tag="ob")
                    lsl = slice(ls * 128, (ls + 1) * 128)
                    for dmc in range(2):
                        pf = ppool.tile([128, LCH], F32, tag="proj",
                                        name=f"fin_{lc}_{ls}_{dmc}")
                        for hh in range(2):
                            nc.tensor.matmul(
                                pf[:, :],
                                outT[:, hh, lsl],
                                we_sb[:, hh, dmc * 512:(dmc + 1) * 512],
                                start=(hh == 0), stop=(hh == 1),
                            )
                        nc.vector.tensor_copy(
                            ob[:, dmc * 512:(dmc + 1) * 512], pf[:, :]
                        )
                    row0 = lc * LCH + ls * 128
                    nc.sync.dma_start(OUT[row0:row0 + 128, :], ob[:, :])

    nc.compile()
    return nc


def make_in_maps(Q, K, V, Wq, bq, Wk, bk, Wv, We):
    """Per-core input dicts. Core c: batch c//4, head-group c%4."""
    bf = lambda a: np.ascontiguousarray(a).astype(BF16NP)
    qt = [bf(Q[b].T) for b in range(B)]
    kt = [bf(K[b].T) for b in range(B)]
    vt = [bf(V[b].T) for b in range(B)]
    in_maps = []
    for c in range(N_CORES):
        b = c // 4
        g = c % 4
        cs = slice(g * M, (g + 1) * M)
        # We rows stacked two heads deep: row r = 64*(h%2)+e, hh = h//2
        we2 = np.ascontiguousarray(We[cs, :]).reshape(2, 128, D)
        we2 = np.ascontiguousarray(we2.transpose(1, 0, 2))
        in_maps.append({
            "QT": qt[b], "KT": kt[b], "VT": vt[b],
            "WQ": bf(Wq[:, cs]),
            "WK": bf(Wk[:, cs]),
            "WV": bf(Wv[:, cs]),
            "WE": we2.astype(BF16NP),
            "BQ": np.ascontiguousarray(bq[cs]).reshape(2, 128),
            "BK": np.ascontiguousarray(bk[cs]).reshape(2, 128),
        })
    return in_maps


_NC_CACHE = {}


def run(Q, K, V, Wq, bq, Wk, bk, Wv, bv, We, be, trace=False, **build_kw):
    from concourse.bass_utils import run_bass_kernel_spmd

    L = Q.shape[1]
    key = (L, tuple(sorted(build_kw.items())))
    if key not in _NC_CACHE:
        _NC_CACHE[key] = build_program(L=L, S=K.shape[1], **build_kw)
    nc = _NC_CACHE[key]
    in_maps = make_in_maps(Q, K, V, Wq, bq, Wk, bk, Wv, We)
    res = run_bass_kernel_spmd(
        nc, in_maps, core_ids=list(range(N_CORES)), trace=trace
    )
    out = np.zeros((B, L, D), np.float32)
    for c in range(N_CORES):
        out[c // 4] += res.results[c]["OUT"]
    # softmax rows sum to 1 => +bv shifts every attention row by bv;
    # be is a plain output shift.
    out += (bv.astype(np.float64) @ We.astype(np.float64) + be).astype(np.float32)
    return out, res


def kernel(Q, K, V, Wq, bq, Wk, bk, Wv, bv, We, be):
    args = [np.asarray(x, np.float32) for x in
            (Q, K, V, Wq, bq, Wk, bk, Wv, bv, We, be)]
    out, _ = run(*args)
    return out


# revision 15
# speedup vs baseline: 1.4147x; 1.4147x over previous
"""Trainium2 Bass kernel for nn_AttentionLayer (B=2, L=S=2048, D=1024, H=16).

Sharding: batch x head-group. Core c handles batch b=c//4 and heads
[4*(c%4), 4*(c%4)+4). Column-parallel Wq/Wk/Wv, row-parallel We; the
per-core partial outputs are summed on the host (4 partials per batch).

v2 design (vs baseline): all on-chip data bf16 (half DMA + FWL weight
loads), score matmuls row-tiled two heads at a time (K=64 each on row
groups 0/64 -> concurrent), output projection contracts K=128 with two
heads stacked on the partition dim, kT/v held as per-chunk tiles so
attention starts before projections finish, and softmax normalization
uses the DRAM-bounce partition broadcast (no PE/PSUM involvement).
PSUM: sc 2x2 banks + av 2 + proj/fin 2 = 8.

Host folds the zero-cost pieces: bv and be shift every output row by
(bv @ We + be) because softmax rows sum to 1; bq/bk applied on-device.
"""
import sys

for _p in ("/opt/trn_rl_repo", "/root/.axon_site/_ro/trn_rl_repo"):
    if _p not in sys.path:
        sys.path.insert(0, _p)

import ml_dtypes
import numpy as np

import concourse.bass as bass
import concourse.mybir as mybir
from concourse import bacc
from concourse.bass import AP
from concourse.tile import TileContext

F32 = mybir.dt.float32
BF16 = mybir.dt.bfloat16
BF16NP = ml_dtypes.bfloat16

D = 1024          # model dim
H_TOTAL = 16
HG = 4            # heads per core
E = 64            # head dim
M = HG * E        # 256 projected cols per core
DT = D // 128     # 8 d-tiles
LCH = 512         # l-chunk
B = 2
N_CORES = 8


def build_program(L=2048, S=2048, sg=2, sc_bufs=2, ex_bufs=4, stream_bufs=3,
                  ob_bufs=2, o_bufs=2, n_bufs=2, av_bufs=3, proj_bufs=1):
    nc = bacc.Bacc("TRN2")
    QT = nc.dram_tensor("QT", [D, L], BF16, kind="ExternalInput")
    KT = nc.dram_tensor("KT", [D, S], BF16, kind="ExternalInput")
    VT = nc.dram_tensor("VT", [D, S], BF16, kind="ExternalInput")
    WQ = nc.dram_tensor("WQ", [D, M], BF16, kind="ExternalInput")
    WK = nc.dram_tensor("WK", [D, M], BF16, kind="ExternalInput")
    WV = nc.dram_tensor("WV", [D, M], BF16, kind="ExternalInput")
    WE = nc.dram_tensor("WE", [128, 2, D], BF16, kind="ExternalInput")
    BQ = nc.dram_tensor("BQ", [2, 128], F32, kind="ExternalInput")
    BK = nc.dram_tensor("BK", [2, 128], F32, kind="ExternalInput")
    OUT = nc.dram_tensor("OUT", [L, D], F32, kind="ExternalOutput")
    RB = nc.dram_tensor("RB", [(L // LCH) * HG, LCH], F32, kind="Internal")

    n_lch = L // LCH
    n_st = S // 128
    n_ch = S // LCH          # kT/v chunks
    st_per_ch = LCH // 128   # 4 s-tiles per chunk
    n_g = n_st // sg
    EXP = mybir.ActivationFunctionType.Exp

    with TileContext(nc) as tc:
        with tc.tile_pool(name="const", bufs=1) as cpool, \
             tc.tile_pool(name="stream", bufs=stream_bufs) as spool, \
             tc.tile_pool(name="ex", bufs=ex_bufs) as expool, \
             tc.tile_pool(name="norm", bufs=n_bufs) as npool, \
             tc.tile_pool(name="outw", bufs=o_bufs) as opool, \
             tc.tile_pool(name="ob", bufs=ob_bufs) as obpool, \
             tc.tile_pool(name="psc", bufs=sc_bufs, space="PSUM") as scpool, \
             tc.tile_pool(name="pav", bufs=av_bufs, space="PSUM") as avpool, \
             tc.tile_pool(name="pproj", bufs=proj_bufs, space="PSUM") as ppool:

            wq_sb = cpool.tile([128, DT, M], BF16, tag="wq")
            wk_sb = cpool.tile([128, DT, M], BF16, tag="wk")
            wv_sb = cpool.tile([128, DT, M], BF16, tag="wv")
            we_sb = cpool.tile([128, 2, D], BF16, tag="we")
            bq_sb = cpool.tile([128, 2], F32, tag="bq")
            bk_sb = cpool.tile([128, 2], F32, tag="bk")
            nc.sync.dma_start(wq_sb[:, :, :], WQ.rearrange("(t p) m -> p t m", p=128))
            nc.sync.dma_start(wk_sb[:, :, :], WK.rearrange("(t p) m -> p t m", p=128))
            nc.sync.dma_start(wv_sb[:, :, :], WV.rearrange("(t p) m -> p t m", p=128))
            nc.sync.dma_start(we_sb[:, :, :], WE[:, :, :])
            nc.sync.dma_start(bq_sb[:, :], BQ.rearrange("t p -> p t"))
            nc.sync.dma_start(bk_sb[:, :], BK.rearrange("t p -> p t"))

            # per-l-chunk qT tiles; per-s-chunk kT and v tiles (chunk
            # granularity lets attention start as soon as the first
            # chunks are projected).
            qT_t = [cpool.tile([128, 2, LCH], BF16, tag=f"qT{i}",
                               name=f"qT{i}") for i in range(n_lch)]
            kT_t = [cpool.tile([128, 2, LCH], BF16, tag=f"kT{i}",
                               name=f"kT{i}") for i in range(n_ch)]
            # 96 cols per head: 64 value cols + 32 ones-columns, so the
            # AV matmul lands the softmax denominator replicated on PSUM
            # rows 64:96 (feeds the 32-partition transposed reciprocal).
            v_t = [cpool.tile([128, st_per_ch, HG * 96], BF16, tag=f"v{i}",
                              name=f"v{i}") for i in range(n_ch)]
            for vt in v_t:
                nc.vector.memset(vt[:, :, :], 1.0)

            # ---- projections ----
            def emit_qk_proj_chunk(XT, w_sb, b_sb, dst, c):
                xtr = XT.rearrange("(t p) l -> p t l", p=128)
                ch = spool.tile([128, DT, LCH], BF16, tag="ch")
                nc.sync.dma_start(ch[:, :, :],
                                  xtr[:, :, c * LCH:(c + 1) * LCH])
                for mt in range(2):
                    ps = ppool.tile([128, LCH], F32, tag="proj")
                    for dt in range(DT):
                        nc.tensor.matmul(
                            ps[:, :],
                            w_sb[:, dt, mt * 128:(mt + 1) * 128],
                            ch[:, dt, :],
                            start=(dt == 0), stop=(dt == DT - 1),
                        )
                    nc.vector.tensor_scalar_add(
                        dst[:, mt, :], ps[:, :], b_sb[:, mt:mt + 1],
                    )

            def emit_v_proj_chunk(c):
                vtr = VT.rearrange("(t p) s -> p t s", p=128)
                ch = spool.tile([128, DT, LCH], BF16, tag="ch")
                nc.sync.dma_start(ch[:, :, :],
                                  vtr[:, :, c * LCH:(c + 1) * LCH])
                for st4 in range(st_per_ch):
                    ps = ppool.tile([128, M], F32, tag="proj")
                    for dt in range(DT):
                        nc.tensor.matmul(
                            ps[:, :],
                            ch[:, dt, st4 * 128:(st4 + 1) * 128],
                            wv_sb[:, dt, :],
                            start=(dt == 0), stop=(dt == DT - 1),
                        )
                    dstv = v_t[c][:, st4, :].rearrange(
                        "p (h c) -> p h c", c=96)[:, :, 0:64]
                    srcv = ps.rearrange("p (h c) -> p h c", c=64)
                    nc.vector.tensor_copy(dstv, srcv)

            # Q chunk 0 first (PE warm-up while K/V DMA streams), then
            # K and V interleaved by chunk so attention's s-tile 0..3
            # deps clear early.
            emit_qk_proj_chunk(QT, wq_sb, bq_sb, qT_t[0], 0)
            for c in range(n_ch):
                emit_qk_proj_chunk(KT, wk_sb, bk_sb, kT_t[c], c)
                emit_v_proj_chunk(c)

            # ---- attention + output projection ----
            for lc in range(n_lch):
                # project the NEXT l-chunk's queries here: the scheduler
                # uses these matmuls as PE filler while exp runs
                if lc + 1 < n_lch:
                    emit_qk_proj_chunk(QT, wq_sb, bq_sb, qT_t[lc + 1], lc + 1)
                outT = opool.tile([128, 2, LCH], BF16, tag="outT")
                for hp in range(HG // 2):
                    h0, h1 = 2 * hp, 2 * hp + 1
                    avs = {}
                    for h in (h0, h1):
                        avs[h] = avpool.tile([128, LCH], F32, tag="av",
                                             name=f"av_{lc}_{h}")
                    for st in range(n_st):
                        # one 2-bank unit per s-tile: both heads' scores,
                        # one 1024-elem exp call; double-buffered so
                        # scores(st+1) overlap exp(st)
                        c, s4 = st // st_per_ch, st % st_per_ch
                        sl = slice(s4 * 128, (s4 + 1) * 128)
                        scu = scpool.tile([128, 2, LCH], F32, tag="sc",
                                          name=f"sc_{lc}_{hp}_{st}")
                        # two heads row-tiled: K=64 each on row groups
                        # 0 / 64, concurrent in the array
                        nc.tensor.matmul(
                            scu[:, 0, :],
                            kT_t[c][0:64, hp, sl],
                            qT_t[lc][0:64, hp, :],
                            start=True, stop=True,
                            tile_position=(0, 0),
                        )
                        nc.tensor.matmul(
                            scu[:, 1, :],
                            kT_t[c][64:128, hp, sl],
                            qT_t[lc][64:128, hp, :],
                            start=True, stop=True,
                            tile_position=(64, 0),
                        )
                        ex = expool.tile([128, 2, LCH], BF16, tag="ex",
                                         name=f"ex_{lc}_{hp}_{st}")
                        nc.scalar.activation(
                            ex[:, :, :], scu[:, :, :], EXP,
                            bias=0.0, scale=0.125,
                        )
                        for h in (h0, h1):
                            nc.tensor.matmul(
                                avs[h][0:96, :],
                                v_t[c][:, s4, h * 96:(h + 1) * 96],
                                ex[:, h - h0, :],
                                start=(st == 0),
                                stop=(st == n_st - 1),
                            )
                    for h in (h0, h1):
                        av = avs[h]
                        # denominator sits replicated on av rows 64:96.
                        # 32x32 block-transpose spreads l across 32
                        # partitions: t1[p, b, f] = denom[32b + p], so
                        # reciprocal runs on 16 elems/lane instead of 512.
                        t1 = npool.tile([32, n_st, 32], F32, tag="t1",
                                        name=f"t1_{lc}_{h}")
                        nc.vector.transpose(t1[:, :, :], av[64:96, :])
                        rc = npool.tile([32, n_st], F32, tag="rc",
                                        name=f"rc_{lc}_{h}")
                        nc.vector.reciprocal(rc[:, :], t1[:, :, 0])
                        # bounce through DRAM: write order (p, b) =
                        # RB[16p + b] = 1/denom[32b + p]; broadcast read
                        # back with the matching affine pattern.
                        idx = lc * HG + h
                        rb = RB[idx, :]
                        # scatter-write so DRAM holds recip in l-order:
                        # RB[32b + p] = rc[p, b]
                        nc.sync.dma_start(
                            AP(rb.tensor, rb.offset, [[1, 32], [32, n_st]]),
                            rc[:, :])
                        rep = npool.tile([64, LCH], F32, tag="rep",
                                         name=f"rep_{lc}_{h}")
                        nc.sync.dma_start(
                            rep[:, :],
                            AP(rb.tensor, rb.offset, [[0, 64], [1, LCH]]))
                        p0 = 64 * (h % 2)
                        nc.vector.tensor_mul(
                            outT[p0:p0 + 64, h // 2, :], av[0:64, :],
                            rep[:, :],
                        )
                for ls in range(LCH // 128):
                    ob = obpool.tile([128, D], F32, tag="ob")
                    lsl = slice(ls * 128, (ls + 1) * 128)
                    for dmc in range(2):
                        pf = ppool.tile([128, LCH], F32, tag="proj",
                                        name=f"fin_{lc}_{ls}_{dmc}")
                        for hh in range(2):
                            nc.tensor.matmul(
                                pf[:, :],
                                outT[:, hh, lsl],
                                we_sb[:, hh, dmc * 512:(dmc + 1) * 512],
                                start=(hh == 0), stop=(hh == 1),
                            )
                        nc.vector.tensor_copy(
                            ob[:, dmc * 512:(dmc + 1) * 512], pf[:, :]
                        )
                    row0 = lc * LCH + ls * 128
                    nc.sync.dma_start(OUT[row0:row0 + 128, :], ob[:, :])

    nc.compile()
    return nc


def make_in_maps(Q, K, V, Wq, bq, Wk, bk, Wv, We):
    """Per-core input dicts. Core c: batch c//4, head-group c%4."""
    bf = lambda a: np.ascontiguousarray(a).astype(BF16NP)
    qt = [bf(Q[b].T) for b in range(B)]
    kt = [bf(K[b].T) for b in range(B)]
    vt = [bf(V[b].T) for b in range(B)]
    in_maps = []
    for c in range(N_CORES):
        b = c // 4
        g = c % 4
        cs = slice(g * M, (g + 1) * M)
        # We rows stacked two heads deep: row r = 64*(h%2)+e, hh = h//2
        we2 = np.ascontiguousarray(We[cs, :]).reshape(2, 128, D)
        we2 = np.ascontiguousarray(we2.transpose(1, 0, 2))
        in_maps.append({
            "QT": qt[b], "KT": kt[b], "VT": vt[b],
            "WQ": bf(Wq[:, cs]),
            "WK": bf(Wk[:, cs]),
            "WV": bf(Wv[:, cs]),
            "WE": we2.astype(BF16NP),
            "BQ": np.ascontiguousarray(bq[cs]).reshape(2, 128),
            "BK": np.ascontiguousarray(bk[cs]).reshape(2, 128),
        })
    return in_maps


_NC_CACHE = {}


def run(Q, K, V, Wq, bq, Wk, bk, Wv, bv, We, be, trace=False, **build_kw):
    from concourse.bass_utils import run_bass_kernel_spmd

    L = Q.shape[1]
    key = (L, tuple(sorted(build_kw.items())))
    if key not in _NC_CACHE:
        _NC_CACHE[key] = build_program(L=L, S=K.shape[1], **build_kw)
    nc = _NC_CACHE[key]
    in_maps = make_in_maps(Q, K, V, Wq, bq, Wk, bk, Wv, We)
    res = run_bass_kernel_spmd(
        nc, in_maps, core_ids=list(range(N_CORES)), trace=trace
    )
    out = np.zeros((B, L, D), np.float32)
    for c in range(N_CORES):
        out[c // 4] += res.results[c]["OUT"]
    # softmax rows sum to 1 => +bv shifts every attention row by bv;
    # be is a plain output shift.
    out += (bv.astype(np.float64) @ We.astype(np.float64) + be).astype(np.float32)
    return out, res


def kernel(Q, K, V, Wq, bq, Wk, bk, Wv, bv, We, be):
    args = [np.asarray(x, np.float32) for x in
            (Q, K, V, Wq, bq, Wk, bk, Wv, bv, We, be)]
    out, _ = run(*args)
    return out


# revision 18
# speedup vs baseline: 1.4336x; 1.0134x over previous
"""Trainium2 Bass kernel for nn_AttentionLayer (B=2, L=S=2048, D=1024, H=16).

Sharding: batch x head-group. Core c handles batch b=c//4 and heads
[4*(c%4), 4*(c%4)+4). Column-parallel Wq/Wk/Wv, row-parallel We; the
per-core partial outputs are summed on the host (4 partials per batch).

v2 design (vs baseline): all on-chip data bf16 (half DMA + FWL weight
loads), score matmuls row-tiled two heads at a time (K=64 each on row
groups 0/64 -> concurrent), output projection contracts K=128 with two
heads stacked on the partition dim, kT/v held as per-chunk tiles so
attention starts before projections finish, and softmax normalization
uses the DRAM-bounce partition broadcast (no PE/PSUM involvement).
PSUM: sc 2x2 banks + av 2 + proj/fin 2 = 8.

Host folds the zero-cost pieces: bv and be shift every output row by
(bv @ We + be) because softmax rows sum to 1; bq/bk applied on-device.
"""
import sys

for _p in ("/opt/trn_rl_repo", "/root/.axon_site/_ro/trn_rl_repo"):
    if _p not in sys.path:
        sys.path.insert(0, _p)

import ml_dtypes
import numpy as np

import concourse.bass as bass
import concourse.mybir as mybir
from concourse import bacc
from concourse.bass import AP
from concourse.tile import TileContext

F32 = mybir.dt.float32
BF16 = mybir.dt.bfloat16
BF16NP = ml_dtypes.bfloat16

D = 1024          # model dim
H_TOTAL = 16
HG = 4            # heads per core
E = 64            # head dim
M = HG * E        # 256 projected cols per core
DT = D // 128     # 8 d-tiles
LCH = 512         # l-chunk
B = 2
N_CORES = 8


def build_program(L=2048, S=2048, sg=2, sc_bufs=2, ex_bufs=4, stream_bufs=3,
                  ob_bufs=2, o_bufs=2, n_bufs=2, av_bufs=2, proj_bufs=2):
    nc = bacc.Bacc("TRN2")
    QT = nc.dram_tensor("QT", [D, L], BF16, kind="ExternalInput")
    KT = nc.dram_tensor("KT", [D, S], BF16, kind="ExternalInput")
    VT = nc.dram_tensor("VT", [D, S], BF16, kind="ExternalInput")
    WQ = nc.dram_tensor("WQ", [D, M], BF16, kind="ExternalInput")
    WK = nc.dram_tensor("WK", [D, M], BF16, kind="ExternalInput")
    WV = nc.dram_tensor("WV", [D, M], BF16, kind="ExternalInput")
    WE = nc.dram_tensor("WE", [128, 2, D], BF16, kind="ExternalInput")
    BQ = nc.dram_tensor("BQ", [2, 128], F32, kind="ExternalInput")
    BK = nc.dram_tensor("BK", [2, 128], F32, kind="ExternalInput")
    OUT = nc.dram_tensor("OUT", [L, D], F32, kind="ExternalOutput")
    RB = nc.dram_tensor("RB", [(L // LCH) * HG, LCH], F32, kind="Internal")

    n_lch = L // LCH
    n_st = S // 128
    n_ch = S // LCH          # kT/v chunks
    st_per_ch = LCH // 128   # 4 s-tiles per chunk
    n_g = n_st // sg
    EXP = mybir.ActivationFunctionType.Exp

    with TileContext(nc) as tc:
        with tc.tile_pool(name="const", bufs=1) as cpool, \
             tc.tile_pool(name="stream", bufs=stream_bufs) as spool, \
             tc.tile_pool(name="ex", bufs=ex_bufs) as expool, \
             tc.tile_pool(name="norm", bufs=n_bufs) as npool, \
             tc.tile_pool(name="outw", bufs=o_bufs) as opool, \
             tc.tile_pool(name="ob", bufs=ob_bufs) as obpool, \
             tc.tile_pool(name="psc", bufs=sc_bufs, space="PSUM") as scpool, \
             tc.tile_pool(name="pav", bufs=av_bufs, space="PSUM") as avpool, \
             tc.tile_pool(name="pproj", bufs=proj_bufs, space="PSUM") as ppool:

            wq_sb = cpool.tile([128, DT, M], BF16, tag="wq")
            wk_sb = cpool.tile([128, DT, M], BF16, tag="wk")
            wv_sb = cpool.tile([128, DT, M], BF16, tag="wv")
            we_sb = cpool.tile([128, 2, D], BF16, tag="we")
            bq_sb = cpool.tile([128, 2], F32, tag="bq")
            bk_sb = cpool.tile([128, 2], F32, tag="bk")
            nc.sync.dma_start(wq_sb[:, :, :], WQ.rearrange("(t p) m -> p t m", p=128))
            nc.sync.dma_start(wk_sb[:, :, :], WK.rearrange("(t p) m -> p t m", p=128))
            nc.sync.dma_start(wv_sb[:, :, :], WV.rearrange("(t p) m -> p t m", p=128))
            nc.sync.dma_start(we_sb[:, :, :], WE[:, :, :])
            nc.sync.dma_start(bq_sb[:, :], BQ.rearrange("t p -> p t"))
            nc.sync.dma_start(bk_sb[:, :], BK.rearrange("t p -> p t"))

            # per-l-chunk qT tiles; per-s-chunk kT and v tiles (chunk
            # granularity lets attention start as soon as the first
            # chunks are projected).
            qT_t = [cpool.tile([128, 2, LCH], BF16, tag=f"qT{i}",
                               name=f"qT{i}") for i in range(n_lch)]
            kT_t = [cpool.tile([128, 2, LCH], BF16, tag=f"kT{i}",
                               name=f"kT{i}") for i in range(n_ch)]
            # 96 cols per head: 64 value cols + 32 ones-columns, so the
            # AV matmul lands the softmax denominator replicated on PSUM
            # rows 64:96 (feeds the 32-partition transposed reciprocal).
            v_t = [cpool.tile([128, st_per_ch, HG * 96], BF16, tag=f"v{i}",
                              name=f"v{i}") for i in range(n_ch)]
            for vt in v_t:
                nc.vector.memset(vt[:, :, :], 1.0)

            # ---- projections ----
            def emit_qk_proj_chunk(XT, w_sb, b_sb, dst, c):
                xtr = XT.rearrange("(t p) l -> p t l", p=128)
                ch = spool.tile([128, DT, LCH], BF16, tag="ch")
                nc.sync.dma_start(ch[:, :, :],
                                  xtr[:, :, c * LCH:(c + 1) * LCH])
                for mt in range(2):
                    ps = ppool.tile([128, LCH], F32, tag="proj")
                    for dt in range(DT):
                        nc.tensor.matmul(
                            ps[:, :],
                            w_sb[:, dt, mt * 128:(mt + 1) * 128],
                            ch[:, dt, :],
                            start=(dt == 0), stop=(dt == DT - 1),
                        )
                    nc.vector.tensor_scalar_add(
                        dst[:, mt, :], ps[:, :], b_sb[:, mt:mt + 1],
                    )

            def emit_v_proj_chunk(c):
                vtr = VT.rearrange("(t p) s -> p t s", p=128)
                ch = spool.tile([128, DT, LCH], BF16, tag="ch")
                nc.sync.dma_start(ch[:, :, :],
                                  vtr[:, :, c * LCH:(c + 1) * LCH])
                for st4 in range(st_per_ch):
                    ps = ppool.tile([128, M], F32, tag="proj")
                    for dt in range(DT):
                        nc.tensor.matmul(
                            ps[:, :],
                            ch[:, dt, st4 * 128:(st4 + 1) * 128],
                            wv_sb[:, dt, :],
                            start=(dt == 0), stop=(dt == DT - 1),
                        )
                    dstv = v_t[c][:, st4, :].rearrange(
                        "p (h c) -> p h c", c=96)[:, :, 0:64]
                    srcv = ps.rearrange("p (h c) -> p h c", c=64)
                    nc.vector.tensor_copy(dstv, srcv)

            # Q chunk 0 first (PE warm-up while K/V DMA streams), then
            # K and V interleaved by chunk so attention's s-tile 0..3
            # deps clear early.
            emit_qk_proj_chunk(QT, wq_sb, bq_sb, qT_t[0], 0)
            for c in range(n_ch):
                emit_qk_proj_chunk(KT, wk_sb, bk_sb, kT_t[c], c)
                emit_v_proj_chunk(c)

            # ---- attention + output projection ----
            for lc in range(n_lch):
                outT = opool.tile([128, 2, LCH], BF16, tag="outT")
                for hp in range(HG // 2):
                    h0, h1 = 2 * hp, 2 * hp + 1
                    avs = {}
                    for h in (h0, h1):
                        avs[h] = avpool.tile([128, LCH], F32, tag="av",
                                             name=f"av_{lc}_{h}")
                    for st in range(n_st):
                        # one 2-bank unit per s-tile: both heads' scores,
                        # one 1024-elem exp call; double-buffered so
                        # scores(st+1) overlap exp(st)
                        c, s4 = st // st_per_ch, st % st_per_ch
                        sl = slice(s4 * 128, (s4 + 1) * 128)
                        scu = scpool.tile([128, 2, LCH], F32, tag="sc",
                                          name=f"sc_{lc}_{hp}_{st}")
                        # two heads row-tiled: K=64 each on row groups
                        # 0 / 64, concurrent in the array
                        nc.tensor.matmul(
                            scu[:, 0, :],
                            kT_t[c][0:64, hp, sl],
                            qT_t[lc][0:64, hp, :],
                            start=True, stop=True,
                            tile_position=(0, 0),
                        )
                        nc.tensor.matmul(
                            scu[:, 1, :],
                            kT_t[c][64:128, hp, sl],
                            qT_t[lc][64:128, hp, :],
                            start=True, stop=True,
                            tile_position=(64, 0),
                        )
                        ex = expool.tile([128, 2, LCH], BF16, tag="ex",
                                         name=f"ex_{lc}_{hp}_{st}")
                        nc.scalar.activation(
                            ex[:, :, :], scu[:, :, :], EXP,
                            bias=0.0, scale=0.125,
                        )
                        for h in (h0, h1):
                            nc.tensor.matmul(
                                avs[h][0:96, :],
                                v_t[c][:, s4, h * 96:(h + 1) * 96],
                                ex[:, h - h0, :],
                                start=(st == 0),
                                stop=(st == n_st - 1),
                            )
                    for h in (h0, h1):
                        av = avs[h]
                        # denominator sits replicated on av rows 64:96.
                        # 32x32 block-transpose spreads l across 32
                        # partitions: t1[p, b, f] = denom[32b + p], so
                        # reciprocal runs on 16 elems/lane instead of 512.
                        t1 = npool.tile([32, n_st, 32], F32, tag="t1",
                                        name=f"t1_{lc}_{h}")
                        nc.vector.transpose(t1[:, :, :], av[64:96, :])
                        rc = npool.tile([32, n_st], F32, tag="rc",
                                        name=f"rc_{lc}_{h}")
                        nc.vector.reciprocal(rc[:, :], t1[:, :, 0])
                        # bounce through DRAM: write order (p, b) =
                        # RB[16p + b] = 1/denom[32b + p]; broadcast read
                        # back with the matching affine pattern.
                        idx = lc * HG + h
                        rb = RB[idx, :]
                        # scatter-write so DRAM holds recip in l-order:
                        # RB[32b + p] = rc[p, b]
                        nc.sync.dma_start(
                            AP(rb.tensor, rb.offset, [[1, 32], [32, n_st]]),
                            rc[:, :])
                        rep = npool.tile([64, LCH], F32, tag="rep",
                                         name=f"rep_{lc}_{h}")
                        nc.sync.dma_start(
                            rep[:, :],
                            AP(rb.tensor, rb.offset, [[0, 64], [1, LCH]]))
                        p0 = 64 * (h % 2)
                        nc.vector.tensor_mul(
                            outT[p0:p0 + 64, h // 2, :], av[0:64, :],
                            rep[:, :],
                        )
                # project the NEXT l-chunk's queries BEFORE this chunk's
                # output projection, so attention(lc+1) restarts without
                # waiting for the fin chains
                if lc + 1 < n_lch:
                    emit_qk_proj_chunk(QT, wq_sb, bq_sb, qT_t[lc + 1], lc + 1)
                for ls in range(LCH // 128):
                    ob = obpool.tile([128, D], F32, tag="ob")
                    lsl = slice(ls * 128, (ls + 1) * 128)
                    for dmc in range(2):
                        pf = ppool.tile([128, LCH], F32, tag="proj",
                                        name=f"fin_{lc}_{ls}_{dmc}")
                        for hh in range(2):
                            nc.tensor.matmul(
                                pf[:, :],
                                outT[:, hh, lsl],
                                we_sb[:, hh, dmc * 512:(dmc + 1) * 512],
                                start=(hh == 0), stop=(hh == 1),
                            )
                        nc.vector.tensor_copy(
                            ob[:, dmc * 512:(dmc + 1) * 512], pf[:, :]
                        )
                    row0 = lc * LCH + ls * 128
                    nc.sync.dma_start(OUT[row0:row0 + 128, :], ob[:, :])

    nc.compile()
    return nc


def make_in_maps(Q, K, V, Wq, bq, Wk, bk, Wv, We):
    """Per-core input dicts. Core c: batch c//4, head-group c%4."""
    bf = lambda a: np.ascontiguousarray(a).astype(BF16NP)
    qt = [bf(Q[b].T) for b in range(B)]
    kt = [bf(K[b].T) for b in range(B)]
    vt = [bf(V[b].T) for b in range(B)]
    in_maps = []
    for c in range(N_CORES):
        b = c // 4
        g = c % 4
        cs = slice(g * M, (g + 1) * M)
        # We rows stacked two heads deep: row r = 64*(h%2)+e, hh = h//2
        we2 = np.ascontiguousarray(We[cs, :]).reshape(2, 128, D)
        we2 = np.ascontiguousarray(we2.transpose(1, 0, 2))
        in_maps.append({
            "QT": qt[b], "KT": kt[b], "VT": vt[b],
            "WQ": bf(Wq[:, cs]),
            "WK": bf(Wk[:, cs]),
            "WV": bf(Wv[:, cs]),
            "WE": we2.astype(BF16NP),
            "BQ": np.ascontiguousarray(bq[cs]).reshape(2, 128),
            "BK": np.ascontiguousarray(bk[cs]).reshape(2, 128),
        })
    return in_maps


_NC_CACHE = {}


def run(Q, K, V, Wq, bq, Wk, bk, Wv, bv, We, be, trace=False, **build_kw):
    from concourse.bass_utils import run_bass_kernel_spmd

    L = Q.shape[1]
    key = (L, tuple(sorted(build_kw.items())))
    if key not in _NC_CACHE:
        _NC_CACHE[key] = build_program(L=L, S=K.shape[1], **build_kw)
    nc = _NC_CACHE[key]
    in_maps = make_in_maps(Q, K, V, Wq, bq, Wk, bk, Wv, We)
    res = run_bass_kernel_spmd(
        nc, in_maps, core_ids=list(range(N_CORES)), trace=trace
    )
    out = np.zeros((B, L, D), np.float32)
    for c in range(N_CORES):
        out[c // 4] += res.results[c]["OUT"]
    # softmax rows sum to 1 => +bv shifts every attention row by bv;
    # be is a plain output shift.
    out += (bv.astype(np.float64) @ We.astype(np.float64) + be).astype(np.float32)
    return out, res


def kernel(Q, K, V, Wq, bq, Wk, bk, Wv, bv, We, be):
    args = [np.asarray(x, np.float32) for x in
            (Q, K, V, Wq, bq, Wk, bk, Wv, bv, We, be)]
    out, _ = run(*args)
    return out


# revision 22
# speedup vs baseline: 1.4854x; 1.0361x over previous
"""Trainium2 Bass kernel for nn_AttentionLayer (B=2, L=S=2048, D=1024, H=16).

Sharding: batch x head-group. Core c handles batch b=c//4 and heads
[4*(c%4), 4*(c%4)+4). Column-parallel Wq/Wk/Wv, row-parallel We; the
per-core partial outputs are summed on the host (4 partials per batch).

v2 design (vs baseline): all on-chip data bf16 (half DMA + FWL weight
loads), score matmuls row-tiled two heads at a time (K=64 each on row
groups 0/64 -> concurrent), output projection contracts K=128 with two
heads stacked on the partition dim, kT/v held as per-chunk tiles so
attention starts before projections finish, and softmax normalization
uses the DRAM-bounce partition broadcast (no PE/PSUM involvement).
PSUM: sc 2x2 banks + av 2 + proj/fin 2 = 8.

Host folds the zero-cost pieces: bv and be shift every output row by
(bv @ We + be) because softmax rows sum to 1; bq/bk applied on-device.
"""
import sys

for _p in ("/opt/trn_rl_repo", "/root/.axon_site/_ro/trn_rl_repo"):
    if _p not in sys.path:
        sys.path.insert(0, _p)

import ml_dtypes
import numpy as np

import concourse.bass as bass
import concourse.mybir as mybir
from concourse import bacc
from concourse.bass import AP
from concourse.tile import TileContext

F32 = mybir.dt.float32
BF16 = mybir.dt.bfloat16
BF16NP = ml_dtypes.bfloat16

D = 1024          # model dim
H_TOTAL = 16
HG = 4            # heads per core
E = 64            # head dim
M = HG * E        # 256 projected cols per core
DT = D // 128     # 8 d-tiles
LCH = 512         # l-chunk
B = 2
N_CORES = 8


def build_program(L=2048, S=2048, sg=2, sc_bufs=2, ex_bufs=4, stream_bufs=3,
                  ob_bufs=2, o_bufs=2, n_bufs=2, av_bufs=3, proj_bufs=1):
    nc = bacc.Bacc("TRN2")
    QT = nc.dram_tensor("QT", [D, L], BF16, kind="ExternalInput")
    KT = nc.dram_tensor("KT", [D, S], BF16, kind="ExternalInput")
    VT = nc.dram_tensor("VT", [D, S], BF16, kind="ExternalInput")
    WQ = nc.dram_tensor("WQ", [D, M], BF16, kind="ExternalInput")
    WK = nc.dram_tensor("WK", [D, M], BF16, kind="ExternalInput")
    WV = nc.dram_tensor("WV", [D, M], BF16, kind="ExternalInput")
    WE = nc.dram_tensor("WE", [128, 2, D], BF16, kind="ExternalInput")
    BQ = nc.dram_tensor("BQ", [2, 128], F32, kind="ExternalInput")
    BK = nc.dram_tensor("BK", [2, 128], F32, kind="ExternalInput")
    OUT = nc.dram_tensor("OUT", [L, D], F32, kind="ExternalOutput")
    RB = nc.dram_tensor("RB", [(L // LCH) * HG, LCH], F32, kind="Internal")

    n_lch = L // LCH
    n_st = S // 128
    n_ch = S // LCH          # kT/v chunks
    st_per_ch = LCH // 128   # 4 s-tiles per chunk
    n_g = n_st // sg
    EXP = mybir.ActivationFunctionType.Exp

    with TileContext(nc) as tc:
        with tc.tile_pool(name="const", bufs=1) as cpool, \
             tc.tile_pool(name="stream", bufs=stream_bufs) as spool, \
             tc.tile_pool(name="ex", bufs=ex_bufs) as expool, \
             tc.tile_pool(name="norm", bufs=n_bufs) as npool, \
             tc.tile_pool(name="outw", bufs=o_bufs) as opool, \
             tc.tile_pool(name="ob", bufs=ob_bufs) as obpool, \
             tc.tile_pool(name="psc", bufs=sc_bufs, space="PSUM") as scpool, \
             tc.tile_pool(name="pav", bufs=av_bufs, space="PSUM") as avpool, \
             tc.tile_pool(name="pproj", bufs=proj_bufs, space="PSUM") as ppool:

            wq_sb = cpool.tile([128, DT, M], BF16, tag="wq")
            wk_sb = cpool.tile([128, DT, M], BF16, tag="wk")
            wv_sb = cpool.tile([128, DT, M], BF16, tag="wv")
            we_sb = cpool.tile([128, 2, D], BF16, tag="we")
            bq_sb = cpool.tile([128, 2], F32, tag="bq")
            bk_sb = cpool.tile([128, 2], F32, tag="bk")
            nc.sync.dma_start(wq_sb[:, :, :], WQ.rearrange("(t p) m -> p t m", p=128))
            nc.sync.dma_start(wk_sb[:, :, :], WK.rearrange("(t p) m -> p t m", p=128))
            nc.sync.dma_start(wv_sb[:, :, :], WV.rearrange("(t p) m -> p t m", p=128))
            nc.sync.dma_start(we_sb[:, :, :], WE[:, :, :])
            nc.sync.dma_start(bq_sb[:, :], BQ.rearrange("t p -> p t"))
            nc.sync.dma_start(bk_sb[:, :], BK.rearrange("t p -> p t"))

            # per-l-chunk qT tiles; per-s-chunk kT and v tiles (chunk
            # granularity lets attention start as soon as the first
            # chunks are projected).
            qT_t = [cpool.tile([128, 2, LCH], BF16, tag=f"qT{i}",
                               name=f"qT{i}") for i in range(n_lch)]
            kT_t = [cpool.tile([128, 2, LCH], BF16, tag=f"kT{i}",
                               name=f"kT{i}") for i in range(n_ch)]
            # 96 cols per head: 64 value cols + 32 ones-columns, so the
            # AV matmul lands the softmax denominator replicated on PSUM
            # rows 64:96 (feeds the 32-partition transposed reciprocal).
            v_t = [cpool.tile([128, st_per_ch, HG * 96], BF16, tag=f"v{i}",
                              name=f"v{i}") for i in range(n_ch)]
            for vt in v_t:
                nc.vector.memset(vt[:, :, :], 1.0)

            # ---- projections ----
            def emit_qk_proj_chunk(XT, w_sb, b_sb, dst, c):
                xtr = XT.rearrange("(t p) l -> p t l", p=128)
                ch = spool.tile([128, DT, LCH], BF16, tag="ch")
                nc.sync.dma_start(ch[:, :, :],
                                  xtr[:, :, c * LCH:(c + 1) * LCH])
                for mt in range(2):
                    ps = ppool.tile([128, LCH], F32, tag="proj")
                    for dt in range(DT):
                        nc.tensor.matmul(
                            ps[:, :],
                            w_sb[:, dt, mt * 128:(mt + 1) * 128],
                            ch[:, dt, :],
                            start=(dt == 0), stop=(dt == DT - 1),
                        )
                    nc.vector.tensor_scalar_add(
                        dst[:, mt, :], ps[:, :], b_sb[:, mt:mt + 1],
                    )

            def emit_v_proj_chunk(c):
                vtr = VT.rearrange("(t p) s -> p t s", p=128)
                ch = spool.tile([128, DT, LCH], BF16, tag="ch")
                nc.sync.dma_start(ch[:, :, :],
                                  vtr[:, :, c * LCH:(c + 1) * LCH])
                for st4 in range(st_per_ch):
                    ps = ppool.tile([128, M], F32, tag="proj")
                    for dt in range(DT):
                        nc.tensor.matmul(
                            ps[:, :],
                            ch[:, dt, st4 * 128:(st4 + 1) * 128],
                            wv_sb[:, dt, :],
                            start=(dt == 0), stop=(dt == DT - 1),
                        )
                    dstv = v_t[c][:, st4, :].rearrange(
                        "p (h c) -> p h c", c=96)[:, :, 0:64]
                    srcv = ps.rearrange("p (h c) -> p h c", c=64)
                    nc.vector.tensor_copy(dstv, srcv)

            # Q chunk 0 first (PE warm-up while K/V DMA streams), then
            # K and V interleaved by chunk so attention's s-tile 0..3
            # deps clear early.
            emit_qk_proj_chunk(QT, wq_sb, bq_sb, qT_t[0], 0)
            for c in range(n_ch):
                emit_qk_proj_chunk(KT, wk_sb, bk_sb, kT_t[c], c)
                emit_v_proj_chunk(c)

            # ---- attention + output projection ----
            def make_fin_steps(lc, outT):
                """Deferred outproj for chunk lc: 8 (ls, dmc) steps that
                get woven into the NEXT chunk's st-loop so their norm
                dependency never head-of-line-blocks the PE stream."""
                obs = {}

                def step(k):
                    ls, dmc = k // 2, k % 2
                    if dmc == 0:
                        obs[ls] = obpool.tile([128, D], F32, tag="ob",
                                              name=f"ob_{lc}_{ls}")
                    ob = obs[ls]
                    lsl = slice(ls * 128, (ls + 1) * 128)
                    pf = ppool.tile([128, LCH], F32, tag="proj",
                                    name=f"fin_{lc}_{ls}_{dmc}")
                    for hh in range(2):
                        nc.tensor.matmul(
                            pf[:, :],
                            outT[:, hh, lsl],
                            we_sb[:, hh, dmc * 512:(dmc + 1) * 512],
                            start=(hh == 0), stop=(hh == 1),
                        )
                    nc.vector.tensor_copy(
                        ob[:, dmc * 512:(dmc + 1) * 512], pf[:, :]
                    )
                    if dmc == 1:
                        row0 = lc * LCH + ls * 128
                        nc.sync.dma_start(OUT[row0:row0 + 128, :], ob[:, :])

                return [lambda k=k: step(k) for k in range(8)]

            def make_qproj_steps(lc):
                """Deferred Q projection for chunk lc: DMA now (prefetch),
                two per-mt matmul chains woven into the current chunk."""
                xtr = QT.rearrange("(t p) l -> p t l", p=128)
                ch = spool.tile([128, DT, LCH], BF16, tag="ch",
                                name=f"qch_{lc}")
                nc.sync.dma_start(ch[:, :, :],
                                  xtr[:, :, lc * LCH:(lc + 1) * LCH])

                def step(mt):
                    ps = ppool.tile([128, LCH], F32, tag="proj",
                                    name=f"qp_{lc}_{mt}")
                    for dt in range(DT):
                        nc.tensor.matmul(
                            ps[:, :],
                            wq_sb[:, dt, mt * 128:(mt + 1) * 128],
                            ch[:, dt, :],
                            start=(dt == 0), stop=(dt == DT - 1),
                        )
                    nc.vector.tensor_scalar_add(
                        qT_t[lc][:, mt, :], ps[:, :], bq_sb[:, mt:mt + 1],
                    )

                return [lambda mt=mt: step(mt) for mt in range(2)]

            fin_steps = []
            for lc in range(n_lch):
                outT = opool.tile([128, 2, LCH], BF16, tag="outT")
                qsteps = (make_qproj_steps(lc + 1) if lc + 1 < n_lch else [])
                for hp in range(HG // 2):
                    h0, h1 = 2 * hp, 2 * hp + 1
                    avs = {}
                    pend = None

                    def emit_av(ex, st):
                        c, s4 = st // st_per_ch, st % st_per_ch
                        for h in (h0, h1):
                            if h not in avs:
                                avs[h] = avpool.tile(
                                    [128, LCH], F32, tag="av",
                                    name=f"av_{lc}_{h}")
                            nc.tensor.matmul(
                                avs[h][0:96, :],
                                v_t[c][:, s4, h * 96:(h + 1) * 96],
                                ex[:, h - h0, :],
                                start=(st == 0),
                                stop=(st == n_st - 1),
                            )

                    for st in range(n_st):
                        # one 2-bank unit per s-tile: both heads' scores,
                        # one 1024-elem exp call; double-buffered so
                        # scores(st+1) overlap exp(st)
                        c, s4 = st // st_per_ch, st % st_per_ch
                        sl = slice(s4 * 128, (s4 + 1) * 128)
                        scu = scpool.tile([128, 2, LCH], F32, tag="sc",
                                          name=f"sc_{lc}_{hp}_{st}")
                        # two heads row-tiled: K=64 each on row groups
                        # 0 / 64, concurrent in the array
                        nc.tensor.matmul(
                            scu[:, 0, :],
                            kT_t[c][0:64, hp, sl],
                            qT_t[lc][0:64, hp, :],
                            start=True, stop=True,
                            tile_position=(0, 0),
                        )
                        nc.tensor.matmul(
                            scu[:, 1, :],
                            kT_t[c][64:128, hp, sl],
                            qT_t[lc][64:128, hp, :],
                            start=True, stop=True,
                            tile_position=(64, 0),
                        )
                        ex = expool.tile([128, 2, LCH], BF16, tag="ex",
                                         name=f"ex_{lc}_{hp}_{st}")
                        nc.scalar.activation(
                            ex[:, :, :], scu[:, :, :], EXP,
                            bias=0.0, scale=0.125,
                        )
                        # AV lags one unit so an av-slot wait can never
                        # head-of-line-block the scores stream
                        if pend is not None:
                            emit_av(*pend)
                        pend = (ex, st)
                        # weave deferred work into the ACT-paced slack
                        if hp == 0 and st % 2 == 1 and fin_steps:
                            fin_steps.pop(0)()
                        if hp == 1 and st in (5, 11) and qsteps:
                            qsteps.pop(0)()
                    emit_av(*pend)
                    for h in (h0, h1):
                        av = avs[h]
                        # denominator sits replicated on av rows 64:96.
                        # 32x32 block-transpose spreads l across 32
                        # partitions: t1[p, b, f] = denom[32b + p], so
                        # reciprocal runs on 16 elems/lane instead of 512.
                        t1 = npool.tile([32, n_st, 32], F32, tag="t1",
                                        name=f"t1_{lc}_{h}")
                        nc.vector.transpose(t1[:, :, :], av[64:96, :])
                        rc = npool.tile([32, n_st], F32, tag="rc",
                                        name=f"rc_{lc}_{h}")
                        nc.vector.reciprocal(rc[:, :], t1[:, :, 0])
                        # bounce through DRAM: write order (p, b) =
                        # RB[16p + b] = 1/denom[32b + p]; broadcast read
                        # back with the matching affine pattern.
                        idx = lc * HG + h
                        rb = RB[idx, :]
                        # scatter-write so DRAM holds recip in l-order:
                        # RB[32b + p] = rc[p, b]
                        nc.sync.dma_start(
                            AP(rb.tensor, rb.offset, [[1, 32], [32, n_st]]),
                            rc[:, :])
                        rep = npool.tile([64, LCH], F32, tag="rep",
                                         name=f"rep_{lc}_{h}")
                        nc.sync.dma_start(
                            rep[:, :],
                            AP(rb.tensor, rb.offset, [[0, 64], [1, LCH]]))
                        p0 = 64 * (h % 2)
                        nc.vector.tensor_mul(
                            outT[p0:p0 + 64, h // 2, :], av[0:64, :],
                            rep[:, :],
                        )
                # leftover deferred steps (shouldn't normally remain)
                for f in fin_steps:
                    f()
                for f in qsteps:
                    f()
                fin_steps = make_fin_steps(lc, outT)
            for f in fin_steps:
                f()

    nc.compile()
    return nc


def make_in_maps(Q, K, V, Wq, bq, Wk, bk, Wv, We):
    """Per-core input dicts. Core c: batch c//4, head-group c%4."""
    bf = lambda a: np.ascontiguousarray(a).astype(BF16NP)
    qt = [bf(Q[b].T) for b in range(B)]
    kt = [bf(K[b].T) for b in range(B)]
    vt = [bf(V[b].T) for b in range(B)]
    in_maps = []
    for c in range(N_CORES):
        b = c // 4
        g = c % 4
        cs = slice(g * M, (g + 1) * M)
        # We rows stacked two heads deep: row r = 64*(h%2)+e, hh = h//2
        we2 = np.ascontiguousarray(We[cs, :]).reshape(2, 128, D)
        we2 = np.ascontiguousarray(we2.transpose(1, 0, 2))
        in_maps.append({
            "QT": qt[b], "KT": kt[b], "VT": vt[b],
            "WQ": bf(Wq[:, cs]),
            "WK": bf(Wk[:, cs]),
            "WV": bf(Wv[:, cs]),
            "WE": we2.astype(BF16NP),
            "BQ": np.ascontiguousarray(bq[cs]).reshape(2, 128),
            "BK": np.ascontiguousarray(bk[cs]).reshape(2, 128),
        })
    return in_maps


_NC_CACHE = {}


def run(Q, K, V, Wq, bq, Wk, bk, Wv, bv, We, be, trace=False, **build_kw):
    from concourse.bass_utils import run_bass_kernel_spmd

    L = Q.shape[1]
    key = (L, tuple(sorted(build_kw.items())))
    if key not in _NC_CACHE:
        _NC_CACHE[key] = build_program(L=L, S=K.shape[1], **build_kw)
    nc = _NC_CACHE[key]
    in_maps = make_in_maps(Q, K, V, Wq, bq, Wk, bk, Wv, We)
    res = run_bass_kernel_spmd(
        nc, in_maps, core_ids=list(range(N_CORES)), trace=trace
    )
    out = np.zeros((B, L, D), np.float32)
    for c in range(N_CORES):
        out[c // 4] += res.results[c]["OUT"]
    # softmax rows sum to 1 => +bv shifts every attention row by bv;
    # be is a plain output shift.
    out += (bv.astype(np.float64) @ We.astype(np.float64) + be).astype(np.float32)
    return out, res


def kernel(Q, K, V, Wq, bq, Wk, bk, Wv, bv, We, be):
    args = [np.asarray(x, np.float32) for x in
            (Q, K, V, Wq, bq, Wk, bk, Wv, bv, We, be)]
    out, _ = run(*args)
    return out


# revision 24
# speedup vs baseline: 1.6253x; 1.0941x over previous
"""Trainium2 Bass kernel for nn_AttentionLayer (B=2, L=S=2048, D=1024, H=16).

Sharding: batch x head-group. Core c handles batch b=c//4 and heads
[4*(c%4), 4*(c%4)+4). Column-parallel Wq/Wk/Wv, row-parallel We; the
per-core partial outputs are summed on the host (4 partials per batch).

v2 design (vs baseline): all on-chip data bf16 (half DMA + FWL weight
loads), score matmuls row-tiled two heads at a time (K=64 each on row
groups 0/64 -> concurrent), output projection contracts K=128 with two
heads stacked on the partition dim, kT/v held as per-chunk tiles so
attention starts before projections finish, and softmax normalization
uses the DRAM-bounce partition broadcast (no PE/PSUM involvement).
PSUM: sc 2x2 banks + av 2 + proj/fin 2 = 8.

Host folds the zero-cost pieces: bv and be shift every output row by
(bv @ We + be) because softmax rows sum to 1; bq/bk applied on-device.
"""
import sys

for _p in ("/opt/trn_rl_repo", "/root/.axon_site/_ro/trn_rl_repo"):
    if _p not in sys.path:
        sys.path.insert(0, _p)

import ml_dtypes
import numpy as np

import concourse.bass as bass
import concourse.mybir as mybir
from concourse import bacc
from concourse.bass import AP
from concourse.tile import TileContext

F32 = mybir.dt.float32
BF16 = mybir.dt.bfloat16
BF16NP = ml_dtypes.bfloat16

D = 1024          # model dim
H_TOTAL = 16
HG = 4            # heads per core
E = 64            # head dim
M = HG * E        # 256 projected cols per core
DT = D // 128     # 8 d-tiles
LCH = 512         # l-chunk
B = 2
N_CORES = 8


def build_program(L=2048, S=2048, sg=2, sc_bufs=2, ex_bufs=4, stream_bufs=3,
                  ob_bufs=2, o_bufs=2, n_bufs=2, av_bufs=4):
    nc = bacc.Bacc("TRN2")
    QT = nc.dram_tensor("QT", [D, L], BF16, kind="ExternalInput")
    KT = nc.dram_tensor("KT", [D, S], BF16, kind="ExternalInput")
    VT = nc.dram_tensor("VT", [D, S], BF16, kind="ExternalInput")
    WQ = nc.dram_tensor("WQ", [D, M], BF16, kind="ExternalInput")
    WK = nc.dram_tensor("WK", [D, M], BF16, kind="ExternalInput")
    WV = nc.dram_tensor("WV", [D, M], BF16, kind="ExternalInput")
    WE = nc.dram_tensor("WE", [128, 2, D], BF16, kind="ExternalInput")
    BQ = nc.dram_tensor("BQ", [2, 128], F32, kind="ExternalInput")
    BK = nc.dram_tensor("BK", [2, 128], F32, kind="ExternalInput")
    OUT = nc.dram_tensor("OUT", [L, D], F32, kind="ExternalOutput")
    RB = nc.dram_tensor("RB", [(L // LCH) * HG, LCH], F32, kind="Internal")

    n_lch = L // LCH
    n_st = S // 128
    n_ch = S // LCH          # kT/v chunks
    st_per_ch = LCH // 128   # 4 s-tiles per chunk
    n_g = n_st // sg
    EXP = mybir.ActivationFunctionType.Exp

    with TileContext(nc) as tc:
        with tc.tile_pool(name="const", bufs=1) as cpool, \
             tc.tile_pool(name="stream", bufs=stream_bufs) as spool, \
             tc.tile_pool(name="ex", bufs=ex_bufs) as expool, \
             tc.tile_pool(name="norm", bufs=n_bufs) as npool, \
             tc.tile_pool(name="outw", bufs=o_bufs) as opool, \
             tc.tile_pool(name="ob", bufs=ob_bufs) as obpool, \
             tc.tile_pool(name="psc", bufs=sc_bufs, space="PSUM") as scpool, \
             tc.tile_pool(name="pav", bufs=av_bufs, space="PSUM") as avpool:

            wq_sb = cpool.tile([128, DT, M], BF16, tag="wq")
            wk_sb = cpool.tile([128, DT, M], BF16, tag="wk")
            wv_sb = cpool.tile([128, DT, M], BF16, tag="wv")
            we_sb = cpool.tile([128, 2, D], BF16, tag="we")
            bq_sb = cpool.tile([128, 2], F32, tag="bq")
            bk_sb = cpool.tile([128, 2], F32, tag="bk")
            nc.sync.dma_start(wq_sb[:, :, :], WQ.rearrange("(t p) m -> p t m", p=128))
            nc.sync.dma_start(wk_sb[:, :, :], WK.rearrange("(t p) m -> p t m", p=128))
            nc.sync.dma_start(wv_sb[:, :, :], WV.rearrange("(t p) m -> p t m", p=128))
            nc.sync.dma_start(we_sb[:, :, :], WE[:, :, :])
            nc.sync.dma_start(bq_sb[:, :], BQ.rearrange("t p -> p t"))
            nc.sync.dma_start(bk_sb[:, :], BK.rearrange("t p -> p t"))

            # per-l-chunk qT tiles; per-s-chunk kT and v tiles (chunk
            # granularity lets attention start as soon as the first
            # chunks are projected).
            qT_t = [cpool.tile([128, 2, LCH], BF16, tag=f"qT{i}",
                               name=f"qT{i}") for i in range(n_lch)]
            kT_t = [cpool.tile([128, 2, LCH], BF16, tag=f"kT{i}",
                               name=f"kT{i}") for i in range(n_ch)]
            # 96 cols per head: 64 value cols + 32 ones-columns, so the
            # AV matmul lands the softmax denominator replicated on PSUM
            # rows 64:96 (feeds the 32-partition transposed reciprocal).
            v_t = [cpool.tile([128, st_per_ch, HG * 96], BF16, tag=f"v{i}",
                              name=f"v{i}") for i in range(n_ch)]
            for vt in v_t:
                nc.vector.memset(vt[:, :, :], 1.0)

            # ---- projections ----
            def emit_qk_proj_chunk(XT, w_sb, b_sb, dst, c, who):
                xtr = XT.rearrange("(t p) l -> p t l", p=128)
                ch = spool.tile([128, DT, LCH], BF16, tag="ch",
                                name=f"ch_{who}{c}")
                nc.sync.dma_start(ch[:, :, :],
                                  xtr[:, :, c * LCH:(c + 1) * LCH])
                for mt in range(2):
                    ps = avpool.tile([128, LCH], F32, tag="av",
                                     name=f"ps_{who}{c}_{mt}")
                    for dt in range(DT):
                        nc.tensor.matmul(
                            ps[:, :],
                            w_sb[:, dt, mt * 128:(mt + 1) * 128],
                            ch[:, dt, :],
                            start=(dt == 0), stop=(dt == DT - 1),
                        )
                    nc.vector.tensor_scalar_add(
                        dst[:, mt, :], ps[:, :], b_sb[:, mt:mt + 1],
                    )

            def emit_v_proj_chunk(c):
                vtr = VT.rearrange("(t p) s -> p t s", p=128)
                ch = spool.tile([128, DT, LCH], BF16, tag="ch",
                                name=f"ch_v{c}")
                nc.sync.dma_start(ch[:, :, :],
                                  vtr[:, :, c * LCH:(c + 1) * LCH])
                for st4 in range(st_per_ch):
                    psw = avpool.tile([128, LCH], F32, tag="av",
                                      name=f"ps_v{c}_{st4}")
                    ps = psw[:, 0:M]
                    for dt in range(DT):
                        nc.tensor.matmul(
                            ps[:, :],
                            ch[:, dt, st4 * 128:(st4 + 1) * 128],
                            wv_sb[:, dt, :],
                            start=(dt == 0), stop=(dt == DT - 1),
                        )
                    dstv = v_t[c][:, st4, :].rearrange(
                        "p (h c) -> p h c", c=96)[:, :, 0:64]
                    srcv = ps.rearrange("p (h c) -> p h c", c=64)
                    nc.vector.tensor_copy(dstv, srcv)

            # Q chunk 0 first (PE warm-up while K/V DMA streams), then
            # K and V interleaved by chunk so attention's s-tile 0..3
            # deps clear early.
            emit_qk_proj_chunk(QT, wq_sb, bq_sb, qT_t[0], 0, "q")
            for c in range(n_ch):
                emit_qk_proj_chunk(KT, wk_sb, bk_sb, kT_t[c], c, "k")
                emit_v_proj_chunk(c)

            # ---- attention + output projection ----
            def make_fin_steps(lc, outT):
                """Deferred outproj for chunk lc: 8 (ls, dmc) steps that
                get woven into the NEXT chunk's st-loop so their norm
                dependency never head-of-line-blocks the PE stream."""
                obs = {}

                def step(k):
                    ls, dmc = k // 2, k % 2
                    if dmc == 0:
                        obs[ls] = obpool.tile([128, D], F32, tag="ob",
                                              name=f"ob_{lc}_{ls}")
                    ob = obs[ls]
                    lsl = slice(ls * 128, (ls + 1) * 128)
                    pf = avpool.tile([128, LCH], F32, tag="av",
                                     name=f"fin_{lc}_{ls}_{dmc}")
                    for hh in range(2):
                        nc.tensor.matmul(
                            pf[:, :],
                            outT[:, hh, lsl],
                            we_sb[:, hh, dmc * 512:(dmc + 1) * 512],
                            start=(hh == 0), stop=(hh == 1),
                        )
                    nc.vector.tensor_copy(
                        ob[:, dmc * 512:(dmc + 1) * 512], pf[:, :]
                    )
                    if dmc == 1:
                        row0 = lc * LCH + ls * 128
                        nc.sync.dma_start(OUT[row0:row0 + 128, :], ob[:, :])

                return [lambda k=k: step(k) for k in range(8)]

            def make_qproj_steps(lc):
                """Deferred Q projection for chunk lc: DMA now (prefetch),
                two per-mt matmul chains woven into the current chunk."""
                xtr = QT.rearrange("(t p) l -> p t l", p=128)
                ch = spool.tile([128, DT, LCH], BF16, tag="ch",
                                name=f"qch_{lc}")
                nc.sync.dma_start(ch[:, :, :],
                                  xtr[:, :, lc * LCH:(lc + 1) * LCH])

                def step(mt):
                    ps = avpool.tile([128, LCH], F32, tag="av",
                                     name=f"qp_{lc}_{mt}")
                    for dt in range(DT):
                        nc.tensor.matmul(
                            ps[:, :],
                            wq_sb[:, dt, mt * 128:(mt + 1) * 128],
                            ch[:, dt, :],
                            start=(dt == 0), stop=(dt == DT - 1),
                        )
                    nc.vector.tensor_scalar_add(
                        qT_t[lc][:, mt, :], ps[:, :], bq_sb[:, mt:mt + 1],
                    )

                return [lambda mt=mt: step(mt) for mt in range(2)]

            fin_steps = []
            for lc in range(n_lch):
                outT = opool.tile([128, 2, LCH], BF16, tag="outT")
                qsteps = (make_qproj_steps(lc + 1) if lc + 1 < n_lch else [])
                for hp in range(HG // 2):
                    h0, h1 = 2 * hp, 2 * hp + 1
                    avs = {}
                    pend = None

                    def emit_av(ex, st):
                        c, s4 = st // st_per_ch, st % st_per_ch
                        for h in (h0, h1):
                            if h not in avs:
                                avs[h] = avpool.tile(
                                    [128, LCH], F32, tag="av",
                                    name=f"av_{lc}_{h}")
                            nc.tensor.matmul(
                                avs[h][0:96, :],
                                v_t[c][:, s4, h * 96:(h + 1) * 96],
                                ex[:, h - h0, :],
                                start=(st == 0),
                                stop=(st == n_st - 1),
                            )

                    for st in range(n_st):
                        # one 2-bank unit per s-tile: both heads' scores,
                        # one 1024-elem exp call; double-buffered so
                        # scores(st+1) overlap exp(st)
                        c, s4 = st // st_per_ch, st % st_per_ch
                        sl = slice(s4 * 128, (s4 + 1) * 128)
                        scu = scpool.tile([128, 2, LCH], F32, tag="sc",
                                          name=f"sc_{lc}_{hp}_{st}")
                        # two heads row-tiled: K=64 each on row groups
                        # 0 / 64, concurrent in the array
                        nc.tensor.matmul(
                            scu[:, 0, :],
                            kT_t[c][0:64, hp, sl],
                            qT_t[lc][0:64, hp, :],
                            start=True, stop=True,
                            tile_position=(0, 0),
                        )
                        nc.tensor.matmul(
                            scu[:, 1, :],
                            kT_t[c][64:128, hp, sl],
                            qT_t[lc][64:128, hp, :],
                            start=True, stop=True,
                            tile_position=(64, 0),
                        )
                        ex = expool.tile([128, 2, LCH], BF16, tag="ex",
                                         name=f"ex_{lc}_{hp}_{st}")
                        nc.scalar.activation(
                            ex[:, :, :], scu[:, :, :], EXP,
                            bias=0.0, scale=0.125,
                        )
                        # AV lags one unit so an av-slot wait can never
                        # head-of-line-block the scores stream
                        if pend is not None:
                            emit_av(*pend)
                        pend = (ex, st)
                        # weave deferred work into the ACT-paced slack
                        if (hp == 0 and st >= 3 and st % 2 == 1) or \
                           (hp == 1 and st == 1):
                            if fin_steps:
                                fin_steps.pop(0)()
                        if hp == 1 and st in (5, 11) and qsteps:
                            qsteps.pop(0)()
                    emit_av(*pend)
                    for h in (h0, h1):
                        av = avs[h]
                        # denominator sits replicated on av rows 64:96.
                        # 32x32 block-transpose spreads l across 32
                        # partitions: t1[p, b, f] = denom[32b + p], so
                        # reciprocal runs on 16 elems/lane instead of 512.
                        t1 = npool.tile([32, n_st, 32], F32, tag="t1",
                                        name=f"t1_{lc}_{h}")
                        nc.vector.transpose(t1[:, :, :], av[64:96, :])
                        rc = npool.tile([32, n_st], F32, tag="rc",
                                        name=f"rc_{lc}_{h}")
                        nc.vector.reciprocal(rc[:, :], t1[:, :, 0])
                        # bounce through DRAM: write order (p, b) =
                        # RB[16p + b] = 1/denom[32b + p]; broadcast read
                        # back with the matching affine pattern.
                        idx = lc * HG + h
                        rb = RB[idx, :]
                        # scatter-write so DRAM holds recip in l-order:
                        # RB[32b + p] = rc[p, b]
                        nc.sync.dma_start(
                            AP(rb.tensor, rb.offset, [[1, 32], [32, n_st]]),
                            rc[:, :])
                        rep = npool.tile([64, LCH], F32, tag="rep",
                                         name=f"rep_{lc}_{h}")
                        for q4 in range(4):
                            nc.sync.dma_start(
                                rep[16 * q4:16 * (q4 + 1), :],
                                AP(rb.tensor, rb.offset, [[0, 16], [1, LCH]]))
                        p0 = 64 * (h % 2)
                        nc.vector.tensor_mul(
                            outT[p0:p0 + 64, h // 2, :], av[0:64, :],
                            rep[:, :],
                        )
                # leftover deferred steps (shouldn't normally remain)
                for f in fin_steps:
                    f()
                for f in qsteps:
                    f()
                fin_steps = make_fin_steps(lc, outT)
            for f in fin_steps:
                f()

    nc.compile()
    return nc


def make_in_maps(Q, K, V, Wq, bq, Wk, bk, Wv, We):
    """Per-core input dicts. Core c: batch c//4, head-group c%4."""
    bf = lambda a: np.ascontiguousarray(a).astype(BF16NP)
    qt = [bf(Q[b].T) for b in range(B)]
    kt = [bf(K[b].T) for b in range(B)]
    vt = [bf(V[b].T) for b in range(B)]
    in_maps = []
    for c in range(N_CORES):
        b = c // 4
        g = c % 4
        cs = slice(g * M, (g + 1) * M)
        # We rows stacked two heads deep: row r = 64*(h%2)+e, hh = h//2
        we2 = np.ascontiguousarray(We[cs, :]).reshape(2, 128, D)
        we2 = np.ascontiguousarray(we2.transpose(1, 0, 2))
        in_maps.append({
            "QT": qt[b], "KT": kt[b], "VT": vt[b],
            "WQ": bf(Wq[:, cs]),
            "WK": bf(Wk[:, cs]),
            "WV": bf(Wv[:, cs]),
            "WE": we2.astype(BF16NP),
            "BQ": np.ascontiguousarray(bq[cs]).reshape(2, 128),
            "BK": np.ascontiguousarray(bk[cs]).reshape(2, 128),
        })
    return in_maps


_NC_CACHE = {}


def run(Q, K, V, Wq, bq, Wk, bk, Wv, bv, We, be, trace=False, **build_kw):
    from concourse.bass_utils import run_bass_kernel_spmd

    L = Q.shape[1]
    key = (L, tuple(sorted(build_kw.items())))
    if key not in _NC_CACHE:
        _NC_CACHE[key] = build_program(L=L, S=K.shape[1], **build_kw)
    nc = _NC_CACHE[key]
    in_maps = make_in_maps(Q, K, V, Wq, bq, Wk, bk, Wv, We)
    res = run_bass_kernel_spmd(
        nc, in_maps, core_ids=list(range(N_CORES)), trace=trace
    )
    out = np.zeros((B, L, D), np.float32)
    for c in range(N_CORES):
        out[c // 4] += res.results[c]["OUT"]
    # softmax rows sum to 1 => +bv shifts every attention row by bv;
    # be is a plain output shift.
    out += (bv.astype(np.float64) @ We.astype(np.float64) + be).astype(np.float32)
    return out, res


def kernel(Q, K, V, Wq, bq, Wk, bk, Wv, bv, We, be):
    args = [np.asarray(x, np.float32) for x in
            (Q, K, V, Wq, bq, Wk, bk, Wv, bv, We, be)]
    out, _ = run(*args)
    return out
